# revision 21
# baseline (speedup 1.0000x reference)
"""Trainium2 Bass kernel for nn_BinaryConv2d (binary 3x3 conv + BN + sign).

Reference computation:
    alpha = mean(|w|, axis=(1,2,3))             # per out-channel scale
    y     = conv2d(x, sign(w)*alpha, pad=1)     # NCHW, 3x3, stride 1
    y     = batchnorm(y, batch stats over (N,H,W), eps=1e-5, gamma, beta)
    out   = clip(sign(y), -1, 1)

Distribution: batch dim sharded 8-way across NeuronCores.

Fast path (beta == 0, which holds for the shipped inputs): inside sign()
the BN variance cancels — out = sign(gamma)*sign(y - mean) — and the mean
is LINEAR in x, so the host computes it exactly (float64) from per-channel
plane sums.  The device then runs a single conv pass:

  The 3x3 conv is 3 matmuls per 6-output-row window pair: stationary
  W_j [K=128, M=96] with K=(x-row-block b, in-ch i), M=(out-row r,
  out-ch o), W_j[b*16+i, r*16+o] = sign(w)[o,i,b-r,j].  The moving
  operand is a host-prepared im2col-block tensor [img, b, i, win, W+2]
  (zero padding baked in); tap j streams cols j..j+W-1 and all taps
  accumulate in PSUM.  x is split hi/lo into two bf16 operands (sign
  weights are exact in bf16) so the fp32 conv is reproduced to ~2^-18.
  A single ScalarE Sign activation (per-partition scale/bias) emits the
  output as bf16 (+-1/0 exact); host casts/reshapes to f32 NCHW.

General path (beta != 0): two-pass variant — pass 1 computes conv(x_hi)
+ bn_stats per window, per-channel sums via a tiny fp32 matmul against a
selector, AllReduce([16,2]) across the 8 cores, on-device
scale/bias = f(global stats); pass 2 recomputes the conv exactly (hi+lo)
and applies Sign(scale*y + bias).
"""

import numpy as np
import ml_dtypes

import concourse.bass as bass  # noqa: F401
import concourse.tile as tile
from concourse import bacc, mybir

BF16 = mybir.dt.bfloat16
F32 = mybir.dt.float32
BN_EPS = 1e-5

C = 16          # channels (in == out)
WIN = 6         # output rows per window
B = WIN + 2     # x-row blocks per window
K = B * C       # 128 contraction
M = WIN * C     # 96 psum partitions


def _cfg(n_img, H, W):
    NW = -(-H // WIN)              # windows per image
    PADW = W + 2
    return NW, PADW


# ======================= graph builders =======================

def build_nc_fast(n_img, H, W, n_cores, kwin=8, debug=False):
    """Single-pass graph: conv + per-partition affine Sign -> bf16 out."""
    NW, PADW = _cfg(n_img, H, W)

    nc = bacc.Bacc("TRN2", target_bir_lowering=False, debug=debug,
                   num_devices=n_cores)

    xh = nc.dram_tensor("xh", [n_img, B, C, NW, PADW], BF16, kind="ExternalInput")
    xl = nc.dram_tensor("xl", [n_img, B, C, NW, PADW], BF16, kind="ExternalInput")
    wj = nc.dram_tensor("wj", [3, K, M], BF16, kind="ExternalInput")
    svec = nc.dram_tensor("svec", [M, 2], F32, kind="ExternalInput")
    out = nc.dram_tensor("out", [n_img, WIN, C, NW, W], BF16,
                         kind="ExternalOutput")

    xh_ap, xl_ap, out_ap = xh.ap(), xl.ap(), out.ap()

    with tile.TileContext(nc) as tc:
        with (
            tc.tile_pool(name="consts", bufs=1) as consts,
            tc.tile_pool(name="xin", bufs=3) as xin,
            tc.tile_pool(name="xin2", bufs=3) as xin2,
            tc.tile_pool(name="osb", bufs=3) as osbp,
            tc.tile_pool(name="psum", bufs=8, space="PSUM") as psum,
        ):
            w_sb = consts.tile([K, 3 * M], BF16)
            nc.sync.dma_start(
                w_sb[:].rearrange("k (j m) -> k j m", j=3),
                wj.ap().transpose([1, 0, 2]))
            sv = consts.tile([M, 2], F32)
            nc.sync.dma_start(sv[:], svec.ap())

            def conv_mms(ps, xt, dw, npair, first, last):
                o2 = ps[:].rearrange("p (k c) -> p k c", c=W)[:, 0:npair, :]
                for j in range(3):
                    nc.tensor.matmul(
                        o2, w_sb[:, j * M:(j + 1) * M],
                        xt[:, dw:dw + npair, j:j + W],
                        start=(first and j == 0), stop=(last and j == 2))

            for n in range(n_img):
                for w0 in range(0, NW, kwin):
                    kw = min(kwin, NW - w0)
                    xt = xin.tile([K, kwin, PADW], BF16, tag="xh")
                    nc.sync.dma_start(
                        xt[:, 0:kw, :],
                        xh_ap[n, :, :, w0:w0 + kw, :].rearrange(
                            "b i k c -> (b i) k c"))
                    xt2 = xin2.tile([K, kwin, PADW], BF16, tag="xl")
                    nc.scalar.dma_start(
                        xt2[:, 0:kw, :],
                        xl_ap[n, :, :, w0:w0 + kw, :].rearrange(
                            "b i k c -> (b i) k c"))
                    ob = osbp.tile([M, kwin * W], BF16, tag="ob")
                    for dw in range(0, kw, 2):
                        npair = min(2, kw - dw)
                        ps = psum.tile([M, 2 * W], F32, tag="ps")
                        conv_mms(ps, xt, dw, npair, True, False)
                        conv_mms(ps, xt2, dw, npair, False, True)
                        nc.scalar.activation(
                            ob[:, dw * W:(dw + npair) * W],
                            ps[:, 0:npair * W],
                            func=mybir.ActivationFunctionType.Sign,
                            bias=sv[:, 1:2], scale=sv[:, 0:1])
                    dst = out_ap[n, :, :, w0:w0 + kw, :].rearrange(
                        "r o k c -> (r o) k c")
                    nc.gpsimd.dma_start(
                        dst,
                        ob[:].rearrange("p (k c) -> p k c", c=W)[:, 0:kw, :])

    nc.compile()
    return nc


def build_nc_general(n_img, H, W, n_cores, kwin=8, debug=False):
    """Two-pass graph with on-device BN stats + AllReduce (any beta)."""
    NW, PADW = _cfg(n_img, H, W)
    rem = H - (NW - 1) * WIN       # valid out rows in last window
    n_tot = float(n_cores * n_img * H * W)

    nc = bacc.Bacc("TRN2", target_bir_lowering=False, debug=debug,
                   num_devices=n_cores)

    xh = nc.dram_tensor("xh", [n_img, B, C, NW, PADW], BF16, kind="ExternalInput")
    xl = nc.dram_tensor("xl", [n_img, B, C, NW, PADW], BF16, kind="ExternalInput")
    wj = nc.dram_tensor("wj", [3, K, M], BF16, kind="ExternalInput")
    sel = nc.dram_tensor("sel", [M, C], F32, kind="ExternalInput")
    cvec = nc.dram_tensor("cvec", [C, 4], F32, kind="ExternalInput")
    out = nc.dram_tensor("out", [n_img, WIN, C, NW, W], F32,
                         kind="ExternalOutput")

    xh_ap, xl_ap, out_ap = xh.ap(), xl.ap(), out.ap()

    with tile.TileContext(nc) as tc:
        with (
            tc.tile_pool(name="consts", bufs=1) as consts,
            tc.tile_pool(name="xin", bufs=3) as xin,
            tc.tile_pool(name="xin2", bufs=3) as xin2,
            tc.tile_pool(name="osb", bufs=3) as osbp,
            tc.tile_pool(name="psum", bufs=7, space="PSUM") as psum,
            tc.tile_pool(name="psc", bufs=1, space="PSUM") as pscp,
            tc.tile_pool(name="dram", bufs=1, space="DRAM") as dram,
        ):
            w_sb = consts.tile([K, 3 * M], BF16)
            nc.sync.dma_start(
                w_sb[:].rearrange("k (j m) -> k j m", j=3),
                wj.ap().transpose([1, 0, 2]))
            sel_sb = consts.tile([M, C], F32)
            nc.sync.dma_start(sel_sb[:], sel.ap())
            cv = consts.tile([C, 4], F32)
            nc.sync.dma_start(cv[:], cvec.ap())

            stats_buf = consts.tile([M, n_img * NW * 6], F32)
            if rem < WIN:
                # garbage out-rows of each image's last window would pollute
                # stats (row H touches real x row H-1); zero their slots and
                # bn_stats only the valid partitions there.
                nc.vector.memset(stats_buf[rem * C:M, :], 0.0)

            def conv_mms(ps, xt, dw, npair, first, last):
                o2 = ps[:].rearrange("p (k c) -> p k c", c=W)[:, 0:npair, :]
                for j in range(3):
                    nc.tensor.matmul(
                        o2, w_sb[:, j * M:(j + 1) * M],
                        xt[:, dw:dw + npair, j:j + W],
                        start=(first and j == 0), stop=(last and j == 2))

            # ---- pass 1: statistics ----
            for n in range(n_img):
                for w0 in range(0, NW, kwin):
                    kw = min(kwin, NW - w0)
                    xt = xin.tile([K, kwin, PADW], BF16, tag="xh")
                    nc.sync.dma_start(
                        xt[:, 0:kw, :],
                        xh_ap[n, :, :, w0:w0 + kw, :].rearrange(
                            "b i k c -> (b i) k c"))
                    for dw in range(0, kw, 2):
                        npair = min(2, kw - dw)
                        ps = psum.tile([M, 2 * W], F32, tag="ps")
                        conv_mms(ps, xt, dw, npair, True, True)
                        for p in range(npair):
                            g_idx = n * NW + w0 + dw + p
                            mp = (rem * C if (w0 + dw + p == NW - 1
                                              and rem < WIN) else M)
                            nc.vector.bn_stats(
                                stats_buf[0:mp, g_idx * 6:(g_idx + 1) * 6],
                                ps[0:mp, p * W:(p + 1) * W])

            # ---- stats reduce + allreduce ----
            # bn_stats triples are (count, mean, count*var) x (even, odd);
            # live slots all have count = W/2 and zeroed slots contribute 0,
            # so constant-count conversion works for every slot:
            #   S_p = (W/2)*sum(means);  Q_p = sum(M2) + (W/2)*sum(means^2)
            ns2 = n_img * NW * 2
            sb3 = stats_buf[:].rearrange("p (s t) -> p s t", t=3)
            means = sb3[:, :, 1]
            m2s = sb3[:, :, 2]
            smean = consts.tile([M, 1], F32)
            qa = consts.tile([M, 1], F32)
            qb = consts.tile([M, 1], F32)
            tmpm = consts.tile([M, ns2], F32)
            nc.vector.reduce_sum(smean[:], means, axis=mybir.AxisListType.X)
            nc.vector.reduce_sum(qa[:], m2s, axis=mybir.AxisListType.X)
            nc.vector.tensor_mul(tmpm[:], means, means)
            nc.vector.reduce_sum(qb[:], tmpm[:], axis=mybir.AxisListType.X)
            sums = consts.tile([M, 2], F32)
            half = float(W // 2)
            nc.vector.tensor_scalar_mul(sums[:, 0:1], smean[:], half)
            nc.vector.tensor_scalar_mul(qb[:], qb[:], half)
            nc.vector.tensor_add(sums[:, 1:2], qa[:], qb[:])

            psc = pscp.tile([C, 2], F32)
            nc.tensor.matmul(psc[:], sel_sb[:], sums[:], start=True, stop=True)
            ccin_sb = consts.tile([C, 2], F32)
            nc.vector.tensor_copy(ccin_sb[:], psc[:])

            cc_in = dram.tile([C, 2], F32)
            cc_out = dram.tile([C, 2], F32,
                               addr_space="Shared" if n_cores > 4 else "Local")
            nc.sync.dma_start(cc_in[:], ccin_sb[:])
            nc.gpsimd.collective_compute(
                "AllReduce", mybir.AluOpType.add,
                replica_groups=[list(range(n_cores))],
                ins=[cc_in[:].opt()], outs=[cc_out[:].opt()])
            gsb = consts.tile([C, 2], F32)
            nc.sync.dma_start(gsb[:], cc_out[:])

            # scale = GA*rsqrt(A2*var+eps); bias = BETA - scale*mean
            m_ = consts.tile([C, 1], F32)
            e2 = consts.tile([C, 1], F32)
            t0 = consts.tile([C, 1], F32)
            var_ = consts.tile([C, 1], F32)
            rec = consts.tile([C, 1], F32)
            r_ = consts.tile([C, 1], F32)
            t2 = consts.tile([C, 1], F32)
            sb16 = consts.tile([C, 2], F32)
            nc.vector.tensor_scalar_mul(m_[:], gsb[:, 0:1], 1.0 / n_tot)
            nc.vector.tensor_scalar_mul(e2[:], gsb[:, 1:2], 1.0 / n_tot)
            nc.vector.tensor_mul(t0[:], m_[:], m_[:])
            nc.vector.tensor_sub(var_[:], e2[:], t0[:])
            nc.vector.tensor_mul(var_[:], var_[:], cv[:, 1:2])
            nc.vector.tensor_scalar_add(var_[:], var_[:], BN_EPS)
            nc.vector.reciprocal(rec[:], var_[:])
            nc.scalar.sqrt(r_[:], rec[:])
            nc.vector.tensor_mul(sb16[:, 0:1], cv[:, 2:3], r_[:])
            nc.vector.tensor_mul(t2[:], sb16[:, 0:1], m_[:])
            nc.vector.tensor_sub(sb16[:, 1:2], cv[:, 3:4], t2[:])
            sb96 = consts.tile([M, 2], F32)
            for r in range(WIN):
                nc.sync.dma_start(sb96[r * C:(r + 1) * C, :], sb16[:])

            # ---- pass 2: output ----
            for n in range(n_img):
                for w0 in range(0, NW, kwin):
                    kw = min(kwin, NW - w0)
                    xt = xin.tile([K, kwin, PADW], BF16, tag="xh")
                    nc.sync.dma_start(
                        xt[:, 0:kw, :],
                        xh_ap[n, :, :, w0:w0 + kw, :].rearrange(
                            "b i k c -> (b i) k c"))
                    xt2 = xin2.tile([K, kwin, PADW], BF16, tag="xl")
                    nc.sync.dma_start(
                        xt2[:, 0:kw, :],
                        xl_ap[n, :, :, w0:w0 + kw, :].rearrange(
                            "b i k c -> (b i) k c"))
                    ob = osbp.tile([M, kwin * W], F32, tag="ob")
                    for dw in range(0, kw, 2):
                        npair = min(2, kw - dw)
                        ps = psum.tile([M, 2 * W], F32, tag="ps")
                        conv_mms(ps, xt, dw, npair, True, False)
                        conv_mms(ps, xt2, dw, npair, False, True)
                        nc.scalar.activation(
                            ob[:, dw * W:(dw + npair) * W],
                            ps[:, 0:npair * W],
                            func=mybir.ActivationFunctionType.Sign,
                            bias=sb96[:, 1:2], scale=sb96[:, 0:1])
                    dst = out_ap[n, :, :, w0:w0 + kw, :].rearrange(
                        "r o k c -> (r o) k c")
                    nc.scalar.dma_start(
                        dst,
                        ob[:].rearrange("p (k c) -> p k c", c=W)[:, 0:kw, :])

    nc.compile()
    return nc


# ======================= host-side preparation =======================

def _build_wjs(real_weight):
    sw = np.sign(real_weight).astype(np.float32)
    wjs = np.zeros((3, K, M), np.float32)
    for j in range(3):
        for dh in range(3):
            for r in range(WIN):
                b = r + dh
                wjs[j, b * C:(b + 1) * C, r * C:(r + 1) * C] = sw[:, :, dh, j].T
    return sw, wjs.astype(ml_dtypes.bfloat16)


def _prep_x_shards(x, n_cores, n_img, NW, PADW):
    """Per-core padded bf16 hi/lo im2col-block tensors."""
    _, Cin, H, W = x.shape
    padrows = (NW - 1) * WIN + B - 1 - (H - 1)
    shards = []
    for c in range(n_cores):
        xs = np.asarray(x[c * n_img:(c + 1) * n_img], np.float32)
        xpad = np.zeros((n_img, Cin, H + 1 + padrows, PADW), np.float32)
        xpad[:, :, 1:H + 1, 1:W + 1] = xs
        hi = xpad.astype(ml_dtypes.bfloat16)
        lo = (xpad - hi.astype(np.float32)).astype(ml_dtypes.bfloat16)

        def prep(a):
            s = a.strides
            v = np.lib.stride_tricks.as_strided(
                a, shape=(n_img, Cin, NW, B, PADW),
                strides=(s[0], s[1], WIN * s[2], s[2], s[3]))
            return np.ascontiguousarray(v.transpose(0, 3, 1, 2, 4))
        shards.append((prep(hi), prep(lo)))
    return shards


def host_mean_psum(x, sw):
    """Exact per-channel mean of the UNSCALED conv output via plane sums.

    mean[c] = (1/N) * sum_{i,kh,kw} sw[c,i,kh,kw] * T[i,kh,kw] where
    T is the padded-window sum of x over channel i for that tap.
    """
    N_, Ci, H, W = x.shape
    xs = x.sum(axis=0, dtype=np.float64)          # [C, H, W]
    tot = xs.sum(axis=(1, 2))
    row0, row1 = xs[:, 0, :].sum(axis=1), xs[:, -1, :].sum(axis=1)
    col0, col1 = xs[:, :, 0].sum(axis=1), xs[:, :, -1].sum(axis=1)
    T = np.zeros((Ci, 3, 3), np.float64)
    for kh in range(3):
        dr = kh - 1
        for kw in range(3):
            dc = kw - 1
            t = tot.copy()
            if dr:                     # dr=-1 drops last row, +1 drops first
                t -= row1 if dr == -1 else row0
            if dc:
                t -= col1 if dc == -1 else col0
            if dr and dc:              # excluded row & col overlap in 1 elem
                t += xs[:, -1 if dr == -1 else 0, -1 if dc == -1 else 0]
            T[:, kh, kw] = t
    S = np.einsum("oikl,ikl->o", sw.astype(np.float64), T)
    return S / (N_ * H * W)


def host_prep_fast(x, real_weight, gamma, beta, n_cores):
    N, Cin, H, W = x.shape
    NW, PADW = _cfg(None, H, W)
    n_img = N // n_cores

    sw, wjs = _build_wjs(real_weight)
    alpha = np.mean(np.abs(real_weight), axis=(1, 2, 3)).astype(np.float64)

    mean_psum = host_mean_psum(x, sw)
    gsign = np.sign(gamma).astype(np.float64)
    gsign[alpha == 0] = 0.0                       # y==0 -> sign(beta)=0
    sb16 = np.stack([gsign, -gsign * mean_psum], axis=1).astype(np.float32)
    svec = np.tile(sb16, (WIN, 1))                # [M, 2]

    shards = _prep_x_shards(x, n_cores, n_img, NW, PADW)
    return [{"xh": hi, "xl": lo, "wj": wjs, "svec": svec}
            for hi, lo in shards]


def host_prep_general(x, real_weight, gamma, beta, n_cores):
    N, Cin, H, W = x.shape
    NW, PADW = _cfg(None, H, W)
    n_img = N // n_cores

    sw, wjs = _build_wjs(real_weight)
    alpha = np.mean(np.abs(real_weight), axis=(1, 2, 3)).astype(np.float32)

    selm = np.zeros((M, C), np.float32)
    for r in range(WIN):
        selm[r * C:(r + 1) * C, :] = np.eye(C, dtype=np.float32)
    cvec = np.stack([alpha, alpha * alpha,
                     gamma.astype(np.float32) * alpha,
                     beta.astype(np.float32)], axis=1)

    shards = _prep_x_shards(x, n_cores, n_img, NW, PADW)
    return [{"xh": hi, "xl": lo, "wj": wjs, "sel": selm, "cvec": cvec}
            for hi, lo in shards]


def unpack_out(dev_out, H, W):
    """[n_img, WIN, C, NW, W] (any dtype) -> [n_img, C, H, W] float32."""
    n_img = dev_out.shape[0]
    v = dev_out.transpose(0, 2, 3, 1, 4).reshape(n_img, C, -1, W)
    return np.asarray(v[:, :, 0:H, :], dtype=np.float32)


# ======================= entry point =======================

_NC_CACHE = {}


def _get_nc(kind, key):
    if (kind, key) not in _NC_CACHE:
        n_img, H, W, n_cores = key
        fn = build_nc_fast if kind == "fast" else build_nc_general
        _NC_CACHE[(kind, key)] = fn(n_img, H, W, n_cores)
    return _NC_CACHE[(kind, key)]


def kernel(x, real_weight, gamma, beta):
    from concourse.bass_utils import run_bass_kernel_spmd
    x = np.asarray(x, np.float32)
    real_weight = np.asarray(real_weight, np.float32)
    gamma = np.asarray(gamma, np.float32)
    beta = np.asarray(beta, np.float32)
    n_cores = 8
    n_img = x.shape[0] // n_cores
    H, W = x.shape[2], x.shape[3]
    key = (n_img, H, W, n_cores)

    fast = bool(np.all(beta == 0.0))
    if fast:
        nc = _get_nc("fast", key)
        in_maps = host_prep_fast(x, real_weight, gamma, beta, n_cores)
    else:
        nc = _get_nc("general", key)
        in_maps = host_prep_general(x, real_weight, gamma, beta, n_cores)
    res = run_bass_kernel_spmd(nc, in_maps, core_ids=list(range(n_cores)))
    return np.concatenate([unpack_out(res.results[c]["out"], H, W)
                           for c in range(n_cores)], axis=0)
